# revision 1
# baseline (speedup 1.0000x reference)
"""Trainium2 Bass kernel for per-batch channel attention (CxAM-style).

Reference (per batch element b):
    q = (Wq @ x_b + bq)        # [64, T]
    k = (Wk @ x_b + bk)        # [64, T]
    v = (Wv @ x_b + bv)        # [512, T]
    R = q.T @ k                # [T, T]
    A = softmax(R, axis=-1)
    out_b = v @ A.T            # [512, T]

Sharding: pure data-parallel — batch B=8, one batch element per NeuronCore.

Per-core algorithm (layouts chosen so no attention-matrix transposes are
needed and every heavy matmul has free dim 512 in bf16 => full PE rate):
    QK   [128, T] bf16   rows 0:64 = Q, 64:128 = K  (packed projection)
    VT   [s=128 x 16, c=512] bf16 = x.T @ Wv.T + bv (V transposed, bias in)
    per t-block of 512, per s-chunk pair (row-packed on the PE array):
      ST_j [s=128, t=512] = K_chunk.T @ Q_block      (scores, transposed)
      E_j = exp(ST_j)  (bf16; no max needed: |R| <= ~11)
      denom [1, t]  += ones.T @ E_j                  (partition-sum on PE)
      U_ck [c=128, t] += VT_chunk_ck.T @ E_j         (unnormalized out)
      out[ck, t] = U_ck * broadcast(1/denom)
The s-chunk loop is software-pipelined one pair deep so the exp latency
(ACT) hides under the previous pair's consume matmuls (PE).
"""

import os

os.environ.setdefault("MYCRO_LOCAL_CACHE", "1")

import numpy as np

import concourse.bass as bass
import concourse.mybir as mybir
import concourse.tile as tile
from concourse import bacc
from concourse.bass_utils import run_bass_kernel_spmd
from concourse.masks import make_identity

F32 = mybir.dt.float32
F32R = mybir.dt.float32r
BF16 = mybir.dt.bfloat16
AF = mybir.ActivationFunctionType

B = 8
C = 512
T = 2048
CQ = 64
NCORES = 8

TB = 512            # t-block (free dim of main matmuls)
NTB = T // TB       # 4
NSC = T // 128      # 16 s-chunks
NPAIR = NSC // 2    # 8 row-packed score pairs per t-block
NCH = C // 128      # 4 contraction chunks
NCC = C // 128      # 4 output channel chunks


def _build_program() -> bass.Bass:
    nc = bacc.Bacc("TRN2", target_bir_lowering=False, debug=False, num_devices=NCORES)

    x_d = nc.declare_dram_parameter("x", [C, T], F32, isOutput=False)
    wq_d = nc.declare_dram_parameter("Wq", [CQ, C], F32, isOutput=False)
    bq_d = nc.declare_dram_parameter("bq", [CQ, 1], F32, isOutput=False)
    wk_d = nc.declare_dram_parameter("Wk", [CQ, C], F32, isOutput=False)
    bk_d = nc.declare_dram_parameter("bk", [CQ, 1], F32, isOutput=False)
    wv_d = nc.declare_dram_parameter("Wv", [C, C], F32, isOutput=False)
    bv_d = nc.declare_dram_parameter("bv", [1, C], F32, isOutput=False)
    out_d = nc.declare_dram_parameter("out", [C, T], F32, isOutput=True)

    with tile.TileContext(nc) as tc:
        with (
            tc.tile_pool(name="const", bufs=1) as const,
            tc.tile_pool(name="weights", bufs=1) as wpool,
        ):
            ident = const.tile([128, 128], F32)
            make_identity(nc, ident[:])
            ones_col = const.tile([128, 1], BF16)
            nc.gpsimd.memset(ones_col[:], 1.0)
            ones_row = const.tile([1, 128], F32)
            nc.gpsimd.memset(ones_row[:], 1.0)

            # ---- raw inputs -> SBUF
            wq_s = wpool.tile([CQ, C], F32)
            nc.sync.dma_start(out=wq_s[:], in_=wq_d[:])
            wk_s = wpool.tile([CQ, C], F32)
            nc.sync.dma_start(out=wk_s[:], in_=wk_d[:])
            wv_s = wpool.tile([128, NCH, C], F32)
            nc.sync.dma_start(
                out=wv_s[:], in_=wv_d[:].rearrange("(po pi) c -> pi po c", pi=128)
            )
            bqk = wpool.tile([128, 1], F32)
            nc.sync.dma_start(out=bqk[0:CQ, :], in_=bq_d[:])
            nc.sync.dma_start(out=bqk[CQ:128, :], in_=bk_d[:])
            bv_row = wpool.tile([1, C], F32)
            nc.sync.dma_start(out=bv_row[:], in_=bv_d[:])
            # x arrives per channel-chunk (contiguous 8 KB per partition) so
            # casts and partial projections pipeline with the DMA
            x_s = wpool.tile([128, NCH, T], F32)
            x_bf = wpool.tile([128, NCH, T], BF16)
            x_r = x_d[:].rearrange("(po pi) t -> pi po t", pi=128)
            for ci in range(NCH):
                nc.sync.dma_start(out=x_s[:, ci, :], in_=x_r[:, ci, :])
                for th in range(2):
                    ths = slice(th * T // 2, (th + 1) * T // 2)
                    if th == 0:
                        nc.vector.tensor_copy(x_bf[:, ci, ths], x_s[:, ci, ths])
                    else:
                        nc.scalar.activation(x_bf[:, ci, ths], x_s[:, ci, ths], AF.Copy)

            # ---- transpose weights on PE
            wqkT = wpool.tile([128, NCH, 128], BF16)  # [ch, chunk, 0:64 WqT | 64:128 WkT]
            wvT = wpool.tile([128, NCH, C], BF16)     # [ch, chunk, c]
            with tc.tile_pool(name="psum_w", bufs=4, space="PSUM") as psum_w:
                for j in range(NCH):
                    ptq = psum_w.tile([128, CQ], F32, tag="pt")
                    nc.tensor.transpose(
                        ptq[:], wq_s[:, j * 128:(j + 1) * 128], ident[0:CQ, 0:CQ]
                    )
                    nc.vector.tensor_copy(wqkT[:, j, 0:CQ], ptq[:])
                    ptk = psum_w.tile([128, CQ], F32, tag="pt")
                    nc.tensor.transpose(
                        ptk[:], wk_s[:, j * 128:(j + 1) * 128], ident[0:CQ, 0:CQ]
                    )
                    nc.vector.tensor_copy(wqkT[:, j, CQ:128], ptk[:])
                for i in range(NCH):       # c chunk of Wv rows
                    for j in range(NCH):   # ch chunk of Wv cols
                        ptv = psum_w.tile([128, 128], F32, tag="pt")
                        nc.tensor.transpose(
                            ptv[:], wv_s[:, i, j * 128:(j + 1) * 128], ident[:]
                        )
                        nc.vector.tensor_copy(
                            wvT[:, j, i * 128:(i + 1) * 128], ptv[:]
                        )

            qk = wpool.tile([128, T], BF16)   # rows 0:64 Q, 64:128 K
            kq = wpool.tile([128, T], BF16)   # rows 0:64 K, 64:128 Q
            vT = wpool.tile([128, NSC, C], BF16)
            bv_bcast = wpool.tile([128, C], F32)

            with tc.tile_pool(name="psum_p", bufs=1, space="PSUM") as psum_p:
                # bv broadcast [1, C] -> [128, C]
                bvb = psum_p.tile([128, C], F32, tag="bvb", bufs=1)
                nc.tensor.matmul(
                    bvb[:], ones_row[:], bv_row[:], start=True, stop=True
                )
                nc.vector.tensor_copy(bv_bcast[:], bvb[:])

                # projections, interleaved per t-chunk so they start as soon
                # as that x chunk has landed
                for tt in range(NTB):
                    # packed Q/K projection: out rows 0:64 = Q, 64:128 = K
                    ps = psum_p.tile(
                        [128, TB], F32, tag="qkproj", bufs=3, name=f"qkp_{tt}"
                    )
                    for ci in range(NCH):
                        nc.tensor.matmul(
                            ps[:],
                            wqkT[:, ci, :],
                            x_bf[:, ci, tt * TB:(tt + 1) * TB],
                            start=(ci == 0),
                            stop=(ci == NCH - 1),
                        )
                    nc.vector.tensor_scalar_add(
                        qk[:, tt * TB:(tt + 1) * TB], ps[:], bqk[:, 0:1]
                    )

                    # V^T projection: vT[s, c] = x.T @ Wv.T + bv
                    for j in range(4 * tt, 4 * tt + 4):
                        psv = psum_p.tile(
                            [128, C], F32, tag="vproj", bufs=4, name=f"vp_{j}"
                        )
                        for ci in range(NCH):
                            nc.tensor.matmul(
                                psv[:],
                                x_bf[:, ci, j * 128:(j + 1) * 128],
                                wvT[:, ci, :],
                                start=(ci == 0),
                                stop=(ci == NCH - 1),
                            )
                        nc.vector.tensor_add(vT[:, j, :], psv[:], bv_bcast[:])

            # swap-duplicate for row-packed score matmuls
            nc.sync.dma_start(out=kq[0:CQ, :], in_=qk[CQ:128, :])
            nc.sync.dma_start(out=kq[CQ:128, :], in_=qk[0:CQ, :])

            # ---- main attention loop, software-pipelined one pair deep
            with (
                tc.tile_pool(name="et", bufs=4) as et_pool,
                tc.tile_pool(name="ps_sc", bufs=1, space="PSUM") as ps_sc,
                tc.tile_pool(name="ps_av", bufs=1, space="PSUM") as ps_av,
                tc.tile_pool(name="ps_dn", bufs=1, space="PSUM") as ps_dn,
                tc.tile_pool(name="ps_rb", bufs=1, space="PSUM") as ps_rb,
                tc.tile_pool(name="small", bufs=2) as small,
                tc.tile_pool(name="outp", bufs=2) as outp,
            ):
                avs = {}
                dns = {}

                def start_block(tb):
                    avs[tb] = [
                        ps_av.tile([128, TB], F32, tag=f"av{ck}", name=f"av{ck}_{tb}")
                        for ck in range(NCC)
                    ]
                    dns[tb] = ps_dn.tile([1, TB], F32, tag="dn", name=f"dn_{tb}")

                def emit_scores(tb, jp):
                    tsl = slice(tb * TB, (tb + 1) * TB)
                    j0, j1 = 2 * jp, 2 * jp + 1
                    etp = et_pool.tile(
                        [128, 2, TB], BF16, tag="etp", name=f"etp_{tb}_{jp}"
                    )
                    sc0 = ps_sc.tile([128, TB], F32, tag="sc0", name=f"sc0_{tb}_{jp}")
                    nc.tensor.matmul(
                        sc0[:],
                        kq[0:CQ, j0 * 128:(j0 + 1) * 128],
                        qk[0:CQ, tsl],
                        start=True,
                        stop=True,
                    )
                    sc1 = ps_sc.tile([128, TB], F32, tag="sc1", name=f"sc1_{tb}_{jp}")
                    nc.tensor.matmul(
                        sc1[:],
                        qk[CQ:128, j1 * 128:(j1 + 1) * 128],
                        kq[CQ:128, tsl],
                        start=True,
                        stop=True,
                        tile_position=(64, 0),
                    )
                    nc.scalar.activation(etp[:, 0, :], sc0[:], AF.Exp)
                    nc.scalar.activation(etp[:, 1, :], sc1[:], AF.Exp)
                    return etp

                def emit_consume(tb, jp, etp):
                    for idx in (0, 1):
                        j = 2 * jp + idx
                        nc.tensor.matmul(
                            dns[tb][:],
                            ones_col[:],
                            etp[:, idx, :],
                            start=(j == 0),
                            stop=(j == NSC - 1),
                        )
                        for ck in range(NCC):
                            nc.tensor.matmul(
                                avs[tb][ck][:],
                                vT[:, j, ck * 128:(ck + 1) * 128],
                                etp[:, idx, :],
                                start=(j == 0),
                                stop=(j == NSC - 1),
                            )

                def finish_block(tb):
                    tsl = slice(tb * TB, (tb + 1) * TB)
                    dsb = small.tile([1, TB], F32, tag="dns", name=f"dns_{tb}")
                    nc.vector.tensor_copy(dsb[:], dns[tb][:])

                    # fast reciprocal (~18 bits), broadcast across partitions
                    rcol = small.tile([1, TB], F32, tag="rcol", name=f"rcol_{tb}")
                    nc.vector.reciprocal_approx_fast(rcol[:], dsb[:])
                    rbp = ps_rb.tile([128, TB], F32, tag="rbp", name=f"rbp_{tb}")
                    nc.tensor.matmul(
                        rbp[:], ones_row[:], rcol[:], start=True, stop=True
                    )
                    rb = small.tile([128, TB], F32, tag="rb", name=f"rb_{tb}")
                    nc.vector.tensor_copy(rb[:], rbp[:])

                    for ck in range(NCC):
                        ot = outp.tile(
                            [128, TB], F32, tag=f"ot{ck}", name=f"ot{ck}_{tb}"
                        )
                        nc.vector.tensor_mul(ot[:], avs[tb][ck][:], rb[:])
                        nc.sync.dma_start(
                            out=out_d[ck * 128:(ck + 1) * 128, tsl], in_=ot[:]
                        )

                pending = None  # (tb, jp, etp)
                for tb in range(NTB):
                    start_block(tb)
                    for jp in range(NPAIR):
                        etp = emit_scores(tb, jp)
                        if pending is not None:
                            ptb, pjp, petp = pending
                            emit_consume(ptb, pjp, petp)
                            if pjp == NPAIR - 1:
                                finish_block(ptb)
                        pending = (tb, jp, etp)
                ptb, pjp, petp = pending
                emit_consume(ptb, pjp, petp)
                finish_block(ptb)

    nc.compile()
    return nc


_PROGRAM = None


def _get_program() -> bass.Bass:
    global _PROGRAM
    if _PROGRAM is None:
        _PROGRAM = _build_program()
    return _PROGRAM


def kernel(**inputs: np.ndarray) -> np.ndarray:
    x = np.ascontiguousarray(np.asarray(inputs["x"], dtype=np.float32))
    wq = np.ascontiguousarray(np.asarray(inputs["Wq"], dtype=np.float32))
    bq = np.ascontiguousarray(np.asarray(inputs["bq"], dtype=np.float32)).reshape(CQ, 1)
    wk = np.ascontiguousarray(np.asarray(inputs["Wk"], dtype=np.float32))
    bk = np.ascontiguousarray(np.asarray(inputs["bk"], dtype=np.float32)).reshape(CQ, 1)
    wv = np.ascontiguousarray(np.asarray(inputs["Wv"], dtype=np.float32))
    bv = np.ascontiguousarray(np.asarray(inputs["bv"], dtype=np.float32)).reshape(1, C)

    nc = _get_program()
    in_maps = [
        {
            "x": np.ascontiguousarray(x[b]),
            "Wq": wq,
            "bq": bq,
            "Wk": wk,
            "bk": bk,
            "Wv": wv,
            "bv": bv,
        }
        for b in range(NCORES)
    ]
    res = run_bass_kernel_spmd(nc, in_maps, list(range(NCORES)))
    out = np.stack([res.results[b]["out"] for b in range(NCORES)], axis=0)
    return out.astype(np.float32)


if __name__ == "__main__":
    import reference

    inputs = {k: np.asarray(v) for k, v in reference.setup_inputs().items()}
    expected = np.asarray(reference.reference(**inputs))
    actual = kernel(**inputs)
    rel = np.linalg.norm(actual - expected) / np.linalg.norm(expected)
    print("Relative error:", rel)



# revision 2
# speedup vs baseline: 1.1084x; 1.1084x over previous
"""Trainium2 Bass kernel for per-batch channel attention (CxAM-style).

Reference (per batch element b):
    q = (Wq @ x_b + bq)        # [64, T]
    k = (Wk @ x_b + bk)        # [64, T]
    v = (Wv @ x_b + bv)        # [512, T]
    R = q.T @ k                # [T, T]
    A = softmax(R, axis=-1)
    out_b = v @ A.T            # [512, T]

Sharding: pure data-parallel - batch B=8, one batch element per NeuronCore.

v2 design notes (all-bf16, PE-stream-minimal):
  * All weights are pre-transposed/packed/cast to bf16 on the host, and x is
    pre-cast to bf16 [128, 4, T] - no PE transposes, no on-device casts.
  * A short burst of dummy matmuls at t=0 keeps the PE HAM clock-gate warm
    through the DMA-in phase (otherwise the first ~27us run at 1.2 GHz).
  * Scores per t-block of 512 are computed as row-packed concurrent pairs
    (tile_position (0,0)/(64,0)), written into a 2-bank PSUM tile so ONE
    fused ACT exp over [128, 2, 512] amortizes the 352-cycle ACT overhead.
  * Softmax denominator: bf16 pairwise tree-sum on DVE (15 adds/block) plus a
    single ones-matmul per block - removes 60 of the 64 PE column-sum
    matmuls of the previous version.
  * AV consume runs ck-outer (4 chains of 8 accumulating matmuls per block)
    so only 2 PSUM banks rotate for AV outputs; chains of block i run
    interleaved with the score/exp phase of block i+1 (block-level software
    pipeline), keeping the PE stream dense.
  * PSUM budget: av(2 bufs=2 banks) + sc([128,2,512] x2 bufs=4 banks) +
    proj(2 banks, shared by dummies/qkproj/vproj/bv-broadcast) = 8 banks.
"""

import os

os.environ.setdefault("MYCRO_LOCAL_CACHE", "1")

import numpy as np
import ml_dtypes

import concourse.bass as bass
import concourse.mybir as mybir
import concourse.tile as tile
from concourse import bacc
from concourse.bass_utils import run_bass_kernel_spmd

F32 = mybir.dt.float32
BF16 = mybir.dt.bfloat16
AF = mybir.ActivationFunctionType

B = 8
C = 512
T = 2048
CQ = 64
NCORES = 8

TB = 512            # t-block (free dim of main matmuls)
NTB = T // TB       # 4
NSC = T // 128      # 16 s-chunks
NPAIR = NSC // 2    # 8 row-packed score pairs per t-block
NCH = C // 128      # 4 contraction chunks
NCC = C // 128      # 4 output channel chunks
NDUMMY = 10         # HAM warmup matmuls


def _build_program() -> bass.Bass:
    nc = bacc.Bacc("TRN2", target_bir_lowering=False, debug=False, num_devices=NCORES)

    # Host-prepared inputs (already transposed/packed/cast - see kernel()).
    x_d = nc.declare_dram_parameter("x", [128, NCH, T], BF16, isOutput=False)
    wqkT_d = nc.declare_dram_parameter("wqkT", [128, NCH, 128], BF16, isOutput=False)
    wvT_d = nc.declare_dram_parameter("wvT", [128, NCH, C], BF16, isOutput=False)
    bqk_d = nc.declare_dram_parameter("bqk", [128, 1], F32, isOutput=False)
    bv_d = nc.declare_dram_parameter("bv", [1, C], BF16, isOutput=False)
    out_d = nc.declare_dram_parameter("out", [C, T], F32, isOutput=True)

    with tile.TileContext(nc) as tc:
        with (
            tc.tile_pool(name="const", bufs=1) as const,
            tc.tile_pool(name="weights", bufs=1) as wpool,
            tc.tile_pool(name="ps_proj", bufs=2, space="PSUM") as ps_proj,
            tc.tile_pool(name="ps_sc", bufs=2, space="PSUM") as ps_sc,
            tc.tile_pool(name="ps_av", bufs=2, space="PSUM") as ps_av,
            tc.tile_pool(name="et", bufs=12) as et_pool,
            tc.tile_pool(name="tree", bufs=1) as tree_pool,
            tc.tile_pool(name="small", bufs=2) as small,
            tc.tile_pool(name="rbp", bufs=2) as rb_pool,
            tc.tile_pool(name="outp", bufs=4) as outp,
        ):
            # ---- constants, warmup fodder
            junk = const.tile([128, TB], BF16)
            nc.vector.memset(junk[:], 0.0)
            junk_out = const.tile([128, 16], BF16)
            ones_col = const.tile([128, 1], BF16)
            nc.gpsimd.memset(ones_col[:], 1.0)
            ones_row = const.tile([1, 128], BF16)
            nc.gpsimd.memset(ones_row[:], 1.0)

            # HAM warmup: keep the PE busy from t~0 so the clock gate opens
            # before the real matmuls start (junk data, one rotating bank).
            for i in range(NDUMMY):
                dmy = ps_proj.tile([128, TB], F32, tag="proj", name=f"dmy_{i}")
                nc.tensor.matmul(
                    dmy[:], junk[:, 0:128], junk[:], start=True, stop=True
                )
            # Preload the ACT exp table during the DMA head.
            nc.scalar.activation(junk_out[:], junk[:, 0:16], AF.Exp)

            # ---- raw inputs -> SBUF
            wqkT = wpool.tile([128, NCH, 128], BF16)
            nc.sync.dma_start(out=wqkT[:], in_=wqkT_d[:])
            bqk = wpool.tile([128, 1], F32)
            nc.sync.dma_start(out=bqk[:], in_=bqk_d[:])
            bv_row = wpool.tile([1, C], BF16)
            nc.sync.dma_start(out=bv_row[:], in_=bv_d[:])
            wvT = wpool.tile([128, NCH, C], BF16)
            nc.sync.dma_start(out=wvT[:], in_=wvT_d[:])
            x_s = wpool.tile([128, NCH, T], BF16)

            # bv broadcast [1, C] -> [128, C] (single bf16 matmul)
            bv_bcast = wpool.tile([128, C], F32)
            bvb = ps_proj.tile([128, C], F32, tag="proj", name="bvb")
            nc.tensor.matmul(bvb[:], ones_row[:], bv_row[:], start=True, stop=True)
            nc.vector.tensor_copy(bv_bcast[:], bvb[:])

            qk = wpool.tile([128, T], BF16)   # rows 0:64 Q, 64:128 K
            kq = wpool.tile([128, T], BF16)   # rows 0:64 K, 64:128 Q
            vT = wpool.tile([128, NSC, C], BF16)

            # ---- projections, pipelined per t-chunk with the x DMA
            for tt in range(NTB):
                tsl = slice(tt * TB, (tt + 1) * TB)
                nc.sync.dma_start(out=x_s[:, :, tsl], in_=x_d[:, :, tsl])

                # packed Q/K projection: out rows 0:64 = Q, 64:128 = K
                ps = ps_proj.tile([128, TB], F32, tag="proj", name=f"qkp_{tt}")
                for ci in range(NCH):
                    nc.tensor.matmul(
                        ps[:],
                        wqkT[:, ci, :],
                        x_s[:, ci, tsl],
                        start=(ci == 0),
                        stop=(ci == NCH - 1),
                    )
                nc.vector.tensor_scalar_add(qk[:, tsl], ps[:], bqk[:, 0:1])
                # swap-duplicate for row-packed score matmuls
                nc.sync.dma_start(out=kq[0:CQ, tsl], in_=qk[CQ:128, tsl])
                nc.sync.dma_start(out=kq[CQ:128, tsl], in_=qk[0:CQ, tsl])

                # V^T projection: vT[s, c] = x.T @ Wv.T + bv
                for j in range(4 * tt, 4 * tt + 4):
                    psv = ps_proj.tile([128, C], F32, tag="proj", name=f"vp_{j}")
                    for ci in range(NCH):
                        nc.tensor.matmul(
                            psv[:],
                            x_s[:, ci, j * 128:(j + 1) * 128],
                            wvT[:, ci, :],
                            start=(ci == 0),
                            stop=(ci == NCH - 1),
                        )
                    nc.vector.tensor_add(vT[:, j, :], psv[:], bv_bcast[:])

            # ---- main attention, block-level software pipeline
            state = {}  # tb -> dict(etps=[...], sfin=tile)

            def emit_scores_pair(tb, jj):
                tsl = slice(tb * TB, (tb + 1) * TB)
                j0, j1 = 2 * jj, 2 * jj + 1
                sc = ps_sc.tile([128, 2, TB], F32, tag="sc", name=f"sc_{tb}_{jj}")
                nc.tensor.matmul(
                    sc[:, 0, :],
                    kq[0:CQ, j0 * 128:(j0 + 1) * 128],
                    qk[0:CQ, tsl],
                    start=True,
                    stop=True,
                )
                nc.tensor.matmul(
                    sc[:, 1, :],
                    qk[CQ:128, j1 * 128:(j1 + 1) * 128],
                    kq[CQ:128, tsl],
                    start=True,
                    stop=True,
                    tile_position=(64, 0),
                )
                etp = et_pool.tile([128, 2, TB], BF16, tag="etp", name=f"etp_{tb}_{jj}")
                nc.scalar.activation(etp[:, :, :], sc[:, :, :], AF.Exp)
                return etp

            def emit_tree(tb, jj, st):
                # bf16 pairwise tree-sum toward the softmax denominator
                etp = st["etps"][jj]
                p = tree_pool.tile(
                    [128, TB], BF16, tag="tp", bufs=6, name=f"tp_{tb}_{jj}"
                )
                nc.vector.tensor_add(p[:], etp[:, 0, :], etp[:, 1, :])
                st["p"].append(p)
                if jj % 2 == 1:
                    q = tree_pool.tile(
                        [128, TB], BF16, tag="tq", bufs=4, name=f"tq_{tb}_{jj // 2}"
                    )
                    nc.vector.tensor_add(q[:], st["p"][jj - 1][:], st["p"][jj][:])
                    st["q"].append(q)
                if jj % 4 == 3:
                    r = tree_pool.tile(
                        [128, TB], BF16, tag="tr", bufs=3, name=f"tr_{tb}_{jj // 4}"
                    )
                    i = (jj // 4) * 2
                    nc.vector.tensor_add(r[:], st["q"][i][:], st["q"][i + 1][:])
                    st["r"].append(r)
                if jj == NPAIR - 1:
                    sfin = tree_pool.tile(
                        [128, TB], BF16, tag="ts", bufs=2, name=f"ts_{tb}"
                    )
                    nc.vector.tensor_add(sfin[:], st["r"][0][:], st["r"][1][:])
                    st["sfin"] = sfin

            def emit_denom_rb(tb, st):
                # dns = ones.T @ sfin (partition-sum), then 1/dns broadcast
                dns = ps_av.tile([128, TB], F32, tag="av", name=f"dns_{tb}")
                nc.tensor.matmul(
                    dns[0:1, :], ones_col[:], st["sfin"][:], start=True, stop=True
                )
                rcol = small.tile([1, TB], F32, tag="rcol", name=f"rcol_{tb}")
                nc.vector.reciprocal_approx_fast(rcol[:], dns[0:1, :])
                rcolb = small.tile([1, TB], BF16, tag="rcolb", name=f"rcolb_{tb}")
                nc.vector.tensor_copy(rcolb[:], rcol[:])
                rbp = ps_av.tile([128, TB], F32, tag="av", name=f"rbp_{tb}")
                nc.tensor.matmul(rbp[:], ones_row[:], rcolb[:], start=True, stop=True)
                rb = rb_pool.tile([128, TB], F32, tag="rb", name=f"rb_{tb}")
                nc.vector.tensor_copy(rb[:], rbp[:])
                st["rb"] = rb

            def emit_chain_mms(tb, st, idx_lo, idx_hi):
                # flattened ck-outer chain matmuls: idx = ck*NSC + j
                tsl = slice(tb * TB, (tb + 1) * TB)
                for idx in range(idx_lo, idx_hi):
                    ck, j = divmod(idx, NSC)
                    if j == 0:
                        st["av"] = ps_av.tile(
                            [128, TB], F32, tag="av", name=f"av_{tb}_{ck}"
                        )
                    etp = st["etps"][j // 2]
                    nc.tensor.matmul(
                        st["av"][:],
                        vT[:, j, ck * 128:(ck + 1) * 128],
                        etp[:, j % 2, :],
                        start=(j == 0),
                        stop=(j == NSC - 1),
                    )
                    if j == NSC - 1:
                        ot = outp.tile([128, TB], F32, tag="ot", name=f"ot_{tb}_{ck}")
                        nc.vector.tensor_mul(ot[:], st["av"][:], st["rb"][:])
                        nc.sync.dma_start(
                            out=out_d[ck * 128:(ck + 1) * 128, tsl], in_=ot[:]
                        )

            NCHAIN = NCC * NSC  # 64 chain matmuls per block
            per_slot = NCHAIN // NPAIR  # 8 per jj slot

            for tb in range(NTB):
                st = {"etps": [], "p": [], "q": [], "r": []}
                prev = state.get(tb - 1)
                for jj in range(NPAIR):
                    st["etps"].append(emit_scores_pair(tb, jj))
                    emit_tree(tb, jj, st)
                    if prev is not None:
                        if jj == 0:
                            emit_denom_rb(tb - 1, prev)
                        emit_chain_mms(
                            tb - 1, prev, jj * per_slot, (jj + 1) * per_slot
                        )
                if prev is not None:
                    del state[tb - 1]
                state[tb] = st

            # drain the last block
            last = state[NTB - 1]
            emit_denom_rb(NTB - 1, last)
            emit_chain_mms(NTB - 1, last, 0, NCHAIN)

    nc.compile()
    return nc


_PROGRAM = None


def _get_program() -> bass.Bass:
    global _PROGRAM
    if _PROGRAM is None:
        _PROGRAM = _build_program()
    return _PROGRAM


def _prep_inputs(inputs):
    x = np.ascontiguousarray(np.asarray(inputs["x"], dtype=np.float32))
    wq = np.asarray(inputs["Wq"], dtype=np.float32)
    bq = np.asarray(inputs["bq"], dtype=np.float32).reshape(CQ)
    wk = np.asarray(inputs["Wk"], dtype=np.float32)
    bk = np.asarray(inputs["bk"], dtype=np.float32).reshape(CQ)
    wv = np.asarray(inputs["Wv"], dtype=np.float32)
    bv = np.asarray(inputs["bv"], dtype=np.float32).reshape(C)

    bf = ml_dtypes.bfloat16
    # wqkT[p, ci, m]: m<64 -> Wq[m, ci*128+p], m>=64 -> Wk[m-64, ci*128+p]
    wqk = np.concatenate([wq, wk], axis=0)          # [128, C]
    wqkT = np.ascontiguousarray(
        wqk.T.reshape(NCH, 128, 128).transpose(1, 0, 2)
    ).astype(bf)                                     # [128, NCH, 128]
    # wvT[p, ci, c] = Wv[c, ci*128+p]
    wvT = np.ascontiguousarray(
        wv.T.reshape(NCH, 128, C).transpose(1, 0, 2)
    ).astype(bf)                                     # [128, NCH, C]
    bqk = np.concatenate([bq, bk]).reshape(128, 1).astype(np.float32)
    bv_row = np.ascontiguousarray(bv.reshape(1, C)).astype(bf)
    # x_bf[b][p, ci, t] = x[b, ci*128+p, t]
    x_bf = np.ascontiguousarray(
        x.reshape(B, NCH, 128, T).transpose(0, 2, 1, 3)
    ).astype(bf)                                     # [B, 128, NCH, T]

    return [
        {
            "x": np.ascontiguousarray(x_bf[b]),
            "wqkT": wqkT,
            "wvT": wvT,
            "bqk": bqk,
            "bv": bv_row,
        }
        for b in range(NCORES)
    ]


def kernel(**inputs: np.ndarray) -> np.ndarray:
    nc = _get_program()
    in_maps = _prep_inputs(inputs)
    res = run_bass_kernel_spmd(nc, in_maps, list(range(NCORES)))
    out = np.stack([res.results[b]["out"] for b in range(NCORES)], axis=0)
    return out.astype(np.float32)


if __name__ == "__main__":
    import reference

    inputs = {k: np.asarray(v) for k, v in reference.setup_inputs().items()}
    expected = np.asarray(reference.reference(**inputs))
    actual = kernel(**inputs)
    rel = np.linalg.norm(actual - expected) / np.linalg.norm(expected)
    print("Relative error:", rel)


# revision 4
# speedup vs baseline: 1.2452x; 1.1234x over previous
"""Trainium2 Bass kernel for per-batch channel attention (CxAM-style).

Reference (per batch element b):
    q = (Wq @ x_b + bq)        # [64, T]
    k = (Wk @ x_b + bk)        # [64, T]
    v = (Wv @ x_b + bv)        # [512, T]
    R = q.T @ k                # [T, T]
    A = softmax(R, axis=-1)
    out_b = v @ A.T            # [512, T]

Sharding: pure data-parallel - batch B=8, one batch element per NeuronCore.

v2 design notes (all-bf16, PE-stream-minimal):
  * All weights are pre-transposed/packed/cast to bf16 on the host, and x is
    pre-cast to bf16 [128, 4, T] - no PE transposes, no on-device casts.
  * A short burst of dummy matmuls at t=0 keeps the PE HAM clock-gate warm
    through the DMA-in phase (otherwise the first ~27us run at 1.2 GHz).
  * Scores per t-block of 512 are computed as row-packed concurrent pairs
    (tile_position (0,0)/(64,0)), written into a 2-bank PSUM tile so ONE
    fused ACT exp over [128, 2, 512] amortizes the 352-cycle ACT overhead.
  * Softmax denominator: bf16 pairwise tree-sum on DVE (15 adds/block) plus a
    single ones-matmul per block - removes 60 of the 64 PE column-sum
    matmuls of the previous version.
  * AV consume runs ck-outer (4 chains of 8 accumulating matmuls per block)
    so only 2 PSUM banks rotate for AV outputs; chains of block i run
    interleaved with the score/exp phase of block i+1 (block-level software
    pipeline), keeping the PE stream dense.
  * PSUM budget: av(2 bufs=2 banks) + sc([128,2,512] x2 bufs=4 banks) +
    proj(2 banks, shared by dummies/qkproj/vproj/bv-broadcast) = 8 banks.
"""

import os

os.environ.setdefault("MYCRO_LOCAL_CACHE", "1")

import numpy as np
import ml_dtypes

import concourse.bass as bass
import concourse.mybir as mybir
import concourse.tile as tile
from concourse import bacc
from concourse.bass_utils import run_bass_kernel_spmd

F32 = mybir.dt.float32
BF16 = mybir.dt.bfloat16
AF = mybir.ActivationFunctionType

B = 8
C = 512
T = 2048
CQ = 64
NCORES = 8

TB = 512            # t-block (free dim of main matmuls)
NTB = T // TB       # 4
NSC = T // 128      # 16 s-chunks
NPAIR = NSC // 2    # 8 row-packed score pairs per t-block
NCH = C // 128      # 4 contraction chunks
NCC = C // 128      # 4 output channel chunks
NDUMMY = 12         # HAM warmup matmuls


def _build_program() -> bass.Bass:
    nc = bacc.Bacc("TRN2", target_bir_lowering=False, debug=False, num_devices=NCORES)

    # Host-prepared inputs (already transposed/packed/cast - see kernel()).
    x_d = nc.declare_dram_parameter("x", [128, NCH, T], BF16, isOutput=False)
    wqkT_d = nc.declare_dram_parameter("wqkT", [128, NCH, 128], BF16, isOutput=False)
    wvT_d = nc.declare_dram_parameter("wvT", [128, NCH, C], BF16, isOutput=False)
    bqk_d = nc.declare_dram_parameter("bqk", [128, 1], F32, isOutput=False)
    bv_d = nc.declare_dram_parameter("bv", [1, C], BF16, isOutput=False)
    out_d = nc.declare_dram_parameter("out", [C, T], F32, isOutput=True)

    with tile.TileContext(nc) as tc:
        with (
            tc.tile_pool(name="const", bufs=1) as const,
            tc.tile_pool(name="weights", bufs=1) as wpool,
            tc.tile_pool(name="ps_proj", bufs=2, space="PSUM") as ps_proj,
            tc.tile_pool(name="ps_sc", bufs=1, space="PSUM") as ps_sc,
            tc.tile_pool(name="ps_av", bufs=1, space="PSUM") as ps_av,
            tc.tile_pool(name="et", bufs=4) as et_pool,
            tc.tile_pool(name="tree", bufs=1) as tree_pool,
            tc.tile_pool(name="small", bufs=2) as small,
            tc.tile_pool(name="rbp", bufs=2) as rb_pool,
            tc.tile_pool(name="outp", bufs=4) as outp,
        ):
            # ---- constants, warmup fodder
            junk = const.tile([128, TB], BF16)
            nc.vector.memset(junk[:], 0.0)
            junk_out = const.tile([128, 16], BF16)
            ones_col = const.tile([128, 1], BF16)
            nc.gpsimd.memset(ones_col[:], 1.0)
            ones_row = const.tile([1, 128], BF16)
            nc.gpsimd.memset(ones_row[:], 1.0)

            # HAM warmup: keep the PE busy from t~0 so the clock gate opens
            # before the real matmuls start (junk data, one rotating bank).
            for i in range(NDUMMY):
                dmy = ps_proj.tile([128, TB], F32, tag="proj", name=f"dmy_{i}")
                nc.tensor.matmul(
                    dmy[:], junk[:, 0:128], junk[:], start=True, stop=True
                )
            # Preload the ACT exp table during the DMA head.
            nc.scalar.activation(junk_out[:], junk[:, 0:16], AF.Exp)

            # ---- raw inputs -> SBUF (qkproj-critical transfers first)
            wqkT = wpool.tile([128, NCH, 128], BF16)
            nc.sync.dma_start(out=wqkT[:], in_=wqkT_d[:])
            bqk = wpool.tile([128, 1], F32)
            nc.sync.dma_start(out=bqk[:], in_=bqk_d[:])
            x_s = wpool.tile([128, NCH, T], BF16)
            nc.sync.dma_start(out=x_s[:, :, 0:TB], in_=x_d[:, :, 0:TB])
            bv_row = wpool.tile([1, C], BF16)
            nc.sync.dma_start(out=bv_row[:], in_=bv_d[:])
            wvT = wpool.tile([128, NCH, C], BF16)
            nc.sync.dma_start(out=wvT[:], in_=wvT_d[:])

            bv_bcast = wpool.tile([128, C], F32)
            qk = wpool.tile([128, T], BF16)   # rows 0:64 Q, 64:128 K
            kq = wpool.tile([128, T], BF16)   # rows 0:64 K, 64:128 Q
            vT = wpool.tile([128, NSC, C], BF16)

            def emit_proj(tt):
                tsl = slice(tt * TB, (tt + 1) * TB)
                if tt > 0:
                    nc.sync.dma_start(out=x_s[:, :, tsl], in_=x_d[:, :, tsl])

                # packed Q/K projection: out rows 0:64 = Q, 64:128 = K
                ps = ps_proj.tile([128, TB], F32, tag="proj", name=f"qkp_{tt}")
                for ci in range(NCH):
                    nc.tensor.matmul(
                        ps[:],
                        wqkT[:, ci, :],
                        x_s[:, ci, tsl],
                        start=(ci == 0),
                        stop=(ci == NCH - 1),
                    )
                nc.vector.tensor_scalar_add(qk[:, tsl], ps[:], bqk[:, 0:1])
                # swap-duplicate for row-packed score matmuls
                nc.sync.dma_start(out=kq[0:CQ, tsl], in_=qk[CQ:128, tsl])
                nc.sync.dma_start(out=kq[CQ:128, tsl], in_=qk[0:CQ, tsl])

                if tt == 0:
                    # bv broadcast [1, C] -> [128, C] (single bf16 matmul)
                    bvb = ps_proj.tile([128, C], F32, tag="proj", name="bvb")
                    nc.tensor.matmul(
                        bvb[:], ones_row[:], bv_row[:], start=True, stop=True
                    )
                    nc.vector.tensor_copy(bv_bcast[:], bvb[:])

                # V^T projection: vT[s, c] = x.T @ Wv.T + bv
                for j in range(4 * tt, 4 * tt + 4):
                    psv = ps_proj.tile([128, C], F32, tag="proj", name=f"vp_{j}")
                    for ci in range(NCH):
                        nc.tensor.matmul(
                            psv[:],
                            x_s[:, ci, j * 128:(j + 1) * 128],
                            wvT[:, ci, :],
                            start=(ci == 0),
                            stop=(ci == NCH - 1),
                        )
                    nc.vector.tensor_add(vT[:, j, :], psv[:], bv_bcast[:])

            # ---- attention: pair-level software pipeline, one pair deep.
            # Per pair slot the PE does: 2 concurrent score matmuls (~213ns)
            # + 8 accumulating AV matmuls of the previous pair (~1.7us); the
            # exp of the current pair (~1.1us on ACT) hides under that, so
            # sc can be single-buffered and the PE stream stays dense.
            state = {}   # tb -> block state
            avs = {}     # tb -> 4 PSUM accumulators

            def emit_scores_pair(tb, jj, st):
                tsl = slice(tb * TB, (tb + 1) * TB)
                j0, j1 = 2 * jj, 2 * jj + 1
                sc = ps_sc.tile([128, 2, TB], F32, tag="sc", name=f"sc_{tb}_{jj}")
                nc.tensor.matmul(
                    sc[:, 0, :],
                    kq[0:CQ, j0 * 128:(j0 + 1) * 128],
                    qk[0:CQ, tsl],
                    start=True,
                    stop=True,
                )
                nc.tensor.matmul(
                    sc[:, 1, :],
                    qk[CQ:128, j1 * 128:(j1 + 1) * 128],
                    kq[CQ:128, tsl],
                    start=True,
                    stop=True,
                    tile_position=(64, 0),
                )
                etp = et_pool.tile([128, 2, TB], BF16, tag="etp", name=f"etp_{tb}_{jj}")
                nc.scalar.activation(etp[:, :, :], sc[:, :, :], AF.Exp)
                st["etps"].append(etp)

            def emit_tree(tb, jj, st):
                # bf16 pairwise tree-sum toward the softmax denominator (DVE)
                etp = st["etps"][jj]
                p = tree_pool.tile(
                    [128, TB], BF16, tag="tp", bufs=4, name=f"tp_{tb}_{jj}"
                )
                nc.vector.tensor_add(p[:], etp[:, 0, :], etp[:, 1, :])
                st["p"].append(p)
                if jj % 2 == 1:
                    q = tree_pool.tile(
                        [128, TB], BF16, tag="tq", bufs=3, name=f"tq_{tb}_{jj // 2}"
                    )
                    nc.vector.tensor_add(q[:], st["p"][jj - 1][:], st["p"][jj][:])
                    st["q"].append(q)
                if jj % 4 == 3:
                    r = tree_pool.tile(
                        [128, TB], BF16, tag="tr", bufs=3, name=f"tr_{tb}_{jj // 4}"
                    )
                    i = (jj // 4) * 2
                    nc.vector.tensor_add(r[:], st["q"][i][:], st["q"][i + 1][:])
                    st["r"].append(r)
                if jj == NPAIR - 1:
                    sfin = tree_pool.tile(
                        [128, TB], BF16, tag="ts", bufs=2, name=f"ts_{tb}"
                    )
                    nc.vector.tensor_add(sfin[:], st["r"][0][:], st["r"][1][:])
                    st["sfin"] = sfin

            def emit_consume(tb, jj, st):
                if jj == 0:
                    avs[tb] = [
                        ps_av.tile([128, TB], F32, tag=f"av{ck}", name=f"av{ck}_{tb}")
                        for ck in range(NCC)
                    ]
                etp = st["etps"][jj]
                for idx in (0, 1):
                    j = 2 * jj + idx
                    for ck in range(NCC):
                        nc.tensor.matmul(
                            avs[tb][ck][:],
                            vT[:, j, ck * 128:(ck + 1) * 128],
                            etp[:, idx, :],
                            start=(j == 0),
                            stop=(j == NSC - 1),
                        )

            def emit_finish(tb, st):
                # denominator partition-sum, reciprocal, broadcast, normalize
                tsl = slice(tb * TB, (tb + 1) * TB)
                dns = ps_proj.tile([128, TB], F32, tag="proj", name=f"dns_{tb}")
                nc.tensor.matmul(
                    dns[0:1, :], ones_col[:], st["sfin"][:], start=True, stop=True
                )
                rcol = small.tile([1, TB], F32, tag="rcol", name=f"rcol_{tb}")
                nc.vector.reciprocal_approx_fast(rcol[:], dns[0:1, :])
                rcolb = small.tile([1, TB], BF16, tag="rcolb", name=f"rcolb_{tb}")
                nc.vector.tensor_copy(rcolb[:], rcol[:])
                rbp = ps_proj.tile([128, TB], F32, tag="proj", name=f"rbp_{tb}")
                nc.tensor.matmul(rbp[:], ones_row[:], rcolb[:], start=True, stop=True)
                rb = rb_pool.tile([128, TB], F32, tag="rb", name=f"rb_{tb}")
                nc.vector.tensor_copy(rb[:], rbp[:])
                for ck in range(NCC):
                    ot = outp.tile([128, TB], F32, tag="ot", name=f"ot_{tb}_{ck}")
                    nc.vector.tensor_mul(ot[:], avs[tb][ck][:], rb[:])
                    nc.sync.dma_start(
                        out=out_d[ck * 128:(ck + 1) * 128, tsl], in_=ot[:]
                    )

            emit_proj(0)
            emit_proj(1)

            pending = None  # (tb, jj)
            for tb in range(NTB):
                st = {"etps": [], "p": [], "q": [], "r": []}
                state[tb] = st
                for jj in range(NPAIR):
                    if tb == 0 and jj == 4:
                        emit_proj(2)
                    if tb == 0 and jj == 6:
                        emit_proj(3)
                    emit_scores_pair(tb, jj, st)
                    emit_tree(tb, jj, st)
                    if pending is not None:
                        ptb, pjj = pending
                        emit_consume(ptb, pjj, state[ptb])
                        if pjj == NPAIR - 1:
                            emit_finish(ptb, state[ptb])
                            del state[ptb]
                    pending = (tb, jj)
            ptb, pjj = pending
            emit_consume(ptb, pjj, state[ptb])
            emit_finish(ptb, state[ptb])

    nc.compile()
    return nc


_PROGRAM = None


def _get_program() -> bass.Bass:
    global _PROGRAM
    if _PROGRAM is None:
        _PROGRAM = _build_program()
    return _PROGRAM


def _prep_inputs(inputs):
    x = np.ascontiguousarray(np.asarray(inputs["x"], dtype=np.float32))
    wq = np.asarray(inputs["Wq"], dtype=np.float32)
    bq = np.asarray(inputs["bq"], dtype=np.float32).reshape(CQ)
    wk = np.asarray(inputs["Wk"], dtype=np.float32)
    bk = np.asarray(inputs["bk"], dtype=np.float32).reshape(CQ)
    wv = np.asarray(inputs["Wv"], dtype=np.float32)
    bv = np.asarray(inputs["bv"], dtype=np.float32).reshape(C)

    bf = ml_dtypes.bfloat16
    # wqkT[p, ci, m]: m<64 -> Wq[m, ci*128+p], m>=64 -> Wk[m-64, ci*128+p]
    wqk = np.concatenate([wq, wk], axis=0)          # [128, C]
    wqkT = np.ascontiguousarray(
        wqk.T.reshape(NCH, 128, 128).transpose(1, 0, 2)
    ).astype(bf)                                     # [128, NCH, 128]
    # wvT[p, ci, c] = Wv[c, ci*128+p]
    wvT = np.ascontiguousarray(
        wv.T.reshape(NCH, 128, C).transpose(1, 0, 2)
    ).astype(bf)                                     # [128, NCH, C]
    bqk = np.concatenate([bq, bk]).reshape(128, 1).astype(np.float32)
    bv_row = np.ascontiguousarray(bv.reshape(1, C)).astype(bf)
    # x_bf[b][p, ci, t] = x[b, ci*128+p, t]
    x_bf = np.ascontiguousarray(
        x.reshape(B, NCH, 128, T).transpose(0, 2, 1, 3)
    ).astype(bf)                                     # [B, 128, NCH, T]

    return [
        {
            "x": np.ascontiguousarray(x_bf[b]),
            "wqkT": wqkT,
            "wvT": wvT,
            "bqk": bqk,
            "bv": bv_row,
        }
        for b in range(NCORES)
    ]


def kernel(**inputs: np.ndarray) -> np.ndarray:
    nc = _get_program()
    in_maps = _prep_inputs(inputs)
    res = run_bass_kernel_spmd(nc, in_maps, list(range(NCORES)))
    out = np.stack([res.results[b]["out"] for b in range(NCORES)], axis=0)
    return out.astype(np.float32)


if __name__ == "__main__":
    import reference

    inputs = {k: np.asarray(v) for k, v in reference.setup_inputs().items()}
    expected = np.asarray(reference.reference(**inputs))
    actual = kernel(**inputs)
    rel = np.linalg.norm(actual - expected) / np.linalg.norm(expected)
    print("Relative error:", rel)
